# revision 35
# baseline (speedup 1.0000x reference)
"""Trainium2 Bass kernel for nn_AttentionProbe_80891414053184.

Math (reference):
    y  = relu(x @ W1.T + b1)            # (B,S,H) -> (B,S,128)
    y2 = relu(y @ W2.T + b2)            # (B,S,128)
    l  = y2 @ Wq.T + pos*pos_w  (+mask) # (B,S,8) logits
    p  = softmax(l, axis=S)
    v  = y2 @ Wv.T + bv
    out[b] = sum_{s,h} p*v + bias       # (B,1)

Strategy: sequence-parallel over 8 cores (512 positions x 4 batches = 2048
tokens per core).  Each core streams its x-shard in fp8-e4m3 (W1 pre-scaled
by 64 into e4m3's normal range, the 1/64 folded into W2; measured end-to-end
error vs the fp32 reference ~5e-3), runs layer 1 as DoubleRow matmuls, the
MLP tail per 512-token tile, and emits per-(batch, head, seq-quarter)
partial softmax stats (-max, Z=sum exp, W=sum exp*v) on all 128 partitions.
The host merges the 8x16 partial stats with the standard online-softmax
combine and produces the (4,1) output.

Layout choices that set the critical path:
  * x is host-pre-swizzled to [128, B*KCH, 512] so every DMA is a
    per-partition-contiguous slab AND the stream is tile-major: tile t's
    k-accumulation completes 1/4 of the way into the remaining stream, so
    its relu/W2/head-projection tail overlaps tile t+1's DMA.  Only tile
    3's tail is exposed at the end.
  * Head projections are 32-wide column-tiled matmuls (N=128) that land
    q|v as a (128, 256) psum: lane p = 32*tile + 8*quarter + head.  The
    whole softmax-stats stage then runs on 128 partitions x 128 columns in
    three fused DVE/ACT ops instead of six 512-wide 32-lane ops.
  * Tail operands (y2, Wq/Wv blocks) are bf16: N=128 matmuls run at
    1 cyc/row in bf16 but 4 cyc/row in f32r.
"""

import numpy as np

# Problem dims (hardcoded per harness contract).
B, S, H = 4, 4096, 4096
MLP, NH = 128, 8
NCORES = 8
S_SHARD = S // NCORES        # 512 seq positions per core
TOK = B * S_SHARD            # 2048 tokens per core
NT = TOK // 512              # 4 token tiles of 512 (= one batch each)
KCH = H // 128               # 32 contraction chunks
GRP = 8                      # k-chunks per x DMA slab (0.5 MB fp8)
NG = KCH // GRP              # 4 slabs per tile
QT = 4                       # seq quarters per tile (512 = 4 x 128)
P32 = 32                     # lanes per psum column group (= NT * NH)

W1_SCALE = 64.0              # 2**6: lifts W1 ~N(0, 1/64^2) into e4m3 range

_cache = {}


def _build_nc(h):
    import concourse.mybir as mybir
    import concourse.tile as tile
    from concourse import bacc
    from concourse.tile import add_dep_helper

    f32 = mybir.dt.float32
    f32r = mybir.dt.float32r
    bf16 = mybir.dt.bfloat16
    fp8 = mybir.dt.float8e4
    kch = h // 128

    # Bacc (not bare Bass): its finalize() runs move_matmul_waits_to_ldweights
    # and generate_event_semaphores, which split multi-sem waits to satisfy
    # TRN2's one-wait-per-instruction encoding limit.
    nc = bacc.Bacc()
    xt_d = nc.dram_tensor("xt", [128, NT * kch, 512], fp8,
                          kind="ExternalInput")
    w1_d = nc.dram_tensor("w1s", [128, kch, MLP], fp8, kind="ExternalInput")
    # cwr: [w2t] f32r for the 512-wide layer-2 matmul; cwh: per-tile
    # zero-padded 32-wide head blocks [wq32 x4 | wv32 x4] bf16 for the
    # 128-wide column-tiled head matmuls; cb: [b1 | b2] f32 biases.
    cwr_d = nc.dram_tensor("cwr", [MLP, MLP], f32r, kind="ExternalInput")
    cwh_d = nc.dram_tensor("cwh", [MLP, 2 * P32 * NT], bf16,
                           kind="ExternalInput")
    cb_d = nc.dram_tensor("cb", [MLP, 2], f32, kind="ExternalInput")
    # ca2: [l-add (pos*pos_w + mask) 128 | bv 1] per stats lane.
    ca2_d = nc.dram_tensor("ca2", [128, 128 + 1], f32, kind="ExternalInput")
    st_d = nc.dram_tensor("stats", [128, 3], f32, kind="ExternalOutput")

    AF = mybir.ActivationFunctionType
    AX = mybir.AxisListType
    OP = mybir.AluOpType
    PM = mybir.MatmulPerfMode.DoubleRow

    with tile.TileContext(nc) as tc:
        with (
            tc.tile_pool(name="const", bufs=1) as const,
            tc.tile_pool(name="xp", bufs=1) as xp,
            tc.tile_pool(name="yp", bufs=2) as yp,
            tc.tile_pool(name="y2p", bufs=2) as y2p,
            tc.tile_pool(name="smallp", bufs=1) as smallp,
            tc.tile_pool(name="statsp", bufs=1) as statsp,
            tc.tile_pool(name="ps_y", bufs=4, space="PSUM") as ps_y,
            tc.tile_pool(name="ps_y2", bufs=1, space="PSUM") as ps_y2,
            tc.tile_pool(name="ps_qv", bufs=1, space="PSUM") as ps_qv,
        ):
            # First stream slab on the HWDGE (sync) ring so it starts during
            # the other engines' preamble, ahead of the SWDGE path.
            x_sb = {}
            def x_dma(t, g, eng):
                sl = xp.tile([128, GRP, 512], fp8, tag=f"x{t}_{g}",
                             name=f"x{t}_{g}")
                eng.dma_start(out=sl[:],
                              in_=xt_d[:, t * kch + g * GRP:
                                       t * kch + (g + 1) * GRP, :])
                x_sb[(t, g)] = sl

            w1_sb = const.tile([128, kch, MLP], fp8)
            h2 = kch // 2
            nc.sync.dma_start(out=w1_sb[:, 0:h2, :], in_=w1_d[:, 0:h2, :])
            x_dma(0, 0, nc.gpsimd)
            x_dma(0, 1, nc.gpsimd)
            nc.sync.dma_start(out=w1_sb[:, h2:kch, :], in_=w1_d[:, h2:kch, :])
            ca2_sb = const.tile([128, 128 + 1], f32)
            nc.scalar.dma_start(out=ca2_sb[:], in_=ca2_d[:])
            cwr_sb = const.tile([MLP, MLP], f32r)
            nc.scalar.dma_start(out=cwr_sb[:], in_=cwr_d[:])
            cwh_sb = const.tile([MLP, 2 * P32 * NT], bf16)
            nc.scalar.dma_start(out=cwh_sb[:], in_=cwh_d[:])
            cb_sb = const.tile([MLP, 2], f32)
            nc.scalar.dma_start(out=cb_sb[:], in_=cb_d[:])

            stats_sb = statsp.tile([128, 3], f32)

            # --- Warmup / staging: each engine observes every const-DMA lane
            # once, so steady-state instructions carry at most one new wait
            # (fewer split-events from Bacc's generate_event_semaphores).
            # Only the w1 warmup gates the k-loop; the rest run after it so
            # the first real matmul waits on nothing but w1-half0 + x-slab0.
            warm_ps = ps_y2.tile([128, 512], f32, tag="y2", name="warm_ps")
            warm_pe_last = nc.tensor.matmul(warm_ps[:, 0:NH], w1_sb[:, 0, :],
                                            w1_sb[:, 0, 0:NH],
                                            start=True, stop=True)

            psum_y = []
            for t in range(NT):
                y_ps = ps_y.tile([128, 512], f32, tag="y", name=f"y_ps{t}")
                psum_y.append(y_ps)

            qv_ps = ps_qv.tile([128, 512], f32, tag="qv", name="qv_ps")
            # per-tile stats scratch: strip t uses partitions 32t..32t+32
            # (engines map partition i of in to partition i of out, so the
            # scratch must sit on the same partitions as the psum strip).
            # (tensor_tensor_reduce would fuse the add+max, but that opcode
            # hard-faults this runtime's DVE -- measured, not theoretical.)
            l_sb = smallp.tile([128, 128], f32, tag="l", name="l_sb")
            e_sb = smallp.tile([128, 128], f32, tag="e", name="e_sb")
            ev_sb = smallp.tile([128, 128], f32, tag="ev", name="ev_sb")

            # cwr/cwh/ca2/cb lane warmups (issued early; they only gate the
            # first tail consumers, not the k-loop)
            warm_ps2 = ps_y2.tile([128, 512], f32, tag="y2", name="warm_ps2")
            nc.tensor.matmul(warm_ps2[0:NH, 0:NH], cwr_sb[:, 0:NH],
                             cwr_sb[:, 0:NH], start=True, stop=True)
            warm_ps3 = ps_y2.tile([128, 512], f32, tag="y2", name="warm_ps3")
            nc.tensor.matmul(warm_ps3[0:NH, 0:NH], cwh_sb[:, 0:NH],
                             cwh_sb[:, 0:NH], start=True, stop=True)
            warm_act = const.tile([MLP, 1], f32)
            nc.scalar.copy(out=warm_act[:], in_=cb_sb[:, 0:1])
            warm_act2 = const.tile([128, 1], f32)
            nc.scalar.copy(out=warm_act2[:], in_=ca2_sb[:, 128:129])
            warm_dve = const.tile([128, 1], f32)
            nc.vector.tensor_copy(out=warm_dve[:], in_=ca2_sb[:, 0:1])
            warm_dve2 = const.tile([MLP, 1], f32)
            nc.vector.tensor_copy(out=warm_dve2[:], in_=cb_sb[:, 1:2])

            # PSUM has_written clears are bank-granular, so the 8 logical
            # head-projection groups (4 column strips x q|v) must share ONE
            # accumulation group: a zero-weight matmul opens it (start=True
            # sets has_written over the full region), every head matmul then
            # accumulates with start=False, and the last carries stop=True.
            zw_sb = const.tile([128, 512], bf16)
            nc.vector.memset(zw_sb[:], 0.0)
            # skip_group_check: the sim's zero-region group checker cannot
            # express column-strip accumulation within one bank; the actual
            # per-element has_written semantics (opener sets all bits, strips
            # accumulate) are still simulated and are what hardware does.
            zero_mm = nc.tensor.matmul(qv_ps[:, 0:512], zw_sb[:, 0:128],
                                       zw_sb[:, 0:512], start=True, stop=False,
                                       skip_group_check=True)

            # Tile-major stream: tile t's layer-1 psum completes after its 4
            # slabs, so its MLP tail overlaps tile t+1's stream.  Layer 1 is
            # DoubleRow fp8: one matmul consumes 2 k-chunks.
            def l1_mms(t, g, last_pair=None):
                sl = x_sb[(t, g)]
                pairs = range(0, GRP, 2) if last_pair is None else [last_pair]
                for kk in pairs:
                    k = g * GRP + kk
                    mm = nc.tensor.matmul(
                        psum_y[t][:],
                        w1_sb[:, k:k + 2, :],
                        sl[:, kk:kk + 2, :],
                        start=(k == 0), stop=(k + 2 == kch),
                        perf_mode=PM,
                    )
                    if k == 0 and t == 0:
                        add_dep_helper(mm.ins, warm_pe_last.ins, sync=False,
                                       reason="warmups before first mm")

            def tail(t):
                # two 256-column halves pipeline relu1 (DVE) -> W2 matmul
                # (PE) -> relu2 (ACT), halving the exposed last-tile latency.
                # Each half's W2 psum is its own bank (psum slots are
                # bank-granular) so the two start/stop groups don't collide.
                y_sb = yp.tile([128, 512], f32r, tag="ysb", name=f"y_sb{t}")
                y2_sb = y2p.tile([128, 512], bf16, tag="y2sb",
                                 name=f"y2_sb{t}")
                for hh in range(2):
                    cs = slice(256 * hh, 256 * (hh + 1))
                    nc.vector.tensor_scalar(out=y_sb[:, cs],
                                            in0=psum_y[t][:, cs],
                                            scalar1=cb_sb[:, 0:1],
                                            scalar2=0.0, op0=OP.add,
                                            op1=OP.max)
                    y2_ps = ps_y2.tile([128, 256], f32, tag=f"y2h{hh}",
                                       name=f"y2_ps{t}_{hh}", bufs=1)
                    nc.tensor.matmul(y2_ps[:], cwr_sb[:], y_sb[:, cs],
                                     start=True, stop=True)
                    nc.scalar.activation(out=y2_sb[:, cs], in_=y2_ps[:],
                                         func=AF.Relu,
                                         bias=cb_sb[:, 1:2], scale=1.0)
                # Head projections: block qt is zero outside rows
                # 8qt..8qt+8, and all four quarters of tile t land in ONE
                # 32-partition strip: lane p = 32*t + 8*qt + h.  Tile t's
                # softmax stats then depend only on strip t, so they run
                # inside the stream for tiles 0..2 -- only tile 3's short
                # 32-lane chain is exposed at the end.
                for qt in range(QT):
                    mm = nc.tensor.matmul(
                        qv_ps[32 * t:32 * (t + 1), 0:128],
                        cwh_sb[:, P32 * qt:P32 * (qt + 1)],
                        y2_sb[:, 128 * qt:128 * (qt + 1)],
                        start=False, stop=False,
                        tile_position=(0, 32 * t),
                        skip_group_check=True)
                    if t == 0:
                        add_dep_helper(mm.ins, zero_mm.ins, sync=False,
                                       reason="group opener before accum")
                for qt in range(QT):
                    mm = nc.tensor.matmul(
                        qv_ps[32 * t:32 * (t + 1), 128:256],
                        cwh_sb[:, P32 * NT + P32 * qt:P32 * NT + P32 * (qt + 1)],
                        y2_sb[:, 128 * qt:128 * (qt + 1)],
                        start=False, stop=(qt == QT - 1),
                        tile_position=(0, 32 * t),
                        skip_group_check=True)
                    if t == 0:
                        add_dep_helper(mm.ins, zero_mm.ins, sync=False,
                                       reason="group opener before accum")
                # per-tile softmax stats on strip t (32 lanes x 128 tokens)
                ts32 = slice(32 * t, 32 * (t + 1))
                nc.vector.tensor_add(out=l_sb[ts32, :], in0=qv_ps[ts32, 0:128],
                                     in1=ca2_sb[ts32, 0:128])
                nc.vector.tensor_reduce(out=stats_sb[ts32, 0:1],
                                        in_=l_sb[ts32, :],
                                        axis=AX.X, op=OP.max, negate=True)
                nc.scalar.activation(out=e_sb[ts32, :], in_=l_sb[ts32, :],
                                     func=AF.Exp, bias=stats_sb[ts32, 0:1],
                                     scale=1.0,
                                     accum_out=stats_sb[ts32, 1:2])
                nc.vector.scalar_tensor_tensor(
                    out=ev_sb[ts32, :], in0=qv_ps[ts32, 128:256],
                    scalar=ca2_sb[ts32, 128:129], in1=e_sb[ts32, :],
                    op0=OP.add, op1=OP.mult, accum_out=stats_sb[ts32, 2:3])

            for t in range(NT):
                for g in range(NG):
                    if t == NT - 1 and g == NG - 1:
                        # final slab arrives as pair-sized DMAs so the last
                        # tile's matmuls and tail start mid-slab
                        for kk in range(0, GRP, 2):
                            sl = xp.tile([128, 2, 512], fp8,
                                         tag=f"x{t}_{g}_{kk}",
                                         name=f"x{t}_{g}_{kk}")
                            c0 = t * kch + g * GRP + kk
                            nc.gpsimd.dma_start(out=sl[:],
                                                in_=xt_d[:, c0:c0 + 2, :])
                            k = g * GRP + kk
                            nc.tensor.matmul(
                                psum_y[t][:], w1_sb[:, k:k + 2, :], sl[:],
                                start=False, stop=(k + 2 == kch),
                                perf_mode=PM)
                    else:
                        if (t, g) not in x_sb:
                            x_dma(t, g, nc.gpsimd)
                        l1_mms(t, g)
                tail(t)

            nc.sync.dma_start(out=st_d[:], in_=stats_sb[:])

    nc.finalize()
    return nc


def get_nc(h=H):
    if h not in _cache:
        _cache[h] = _build_nc(h)
    return _cache[h]


def make_core_inputs(x, mask, W1, b1, W2, b2, Wq, Wv, bv, pos_w, bias):
    """Host-side shard + transpose. Returns list of 8 in_maps."""
    import ml_dtypes
    h = x.shape[2]
    kch = h // 128
    # W1 scaled up by 64 into e4m3's normal range; layer-1 output then
    # carries a 64x factor, removed by folding 1/64 into W2 (and 64 into b1,
    # since relu(64a) = 64 relu(a) commutes with the positive scale).
    w1s = np.ascontiguousarray(
        (W1 * W1_SCALE).reshape(MLP, kch, 128).transpose(2, 1, 0)).astype(
            ml_dtypes.float8_e4m3)
    cwr = np.ascontiguousarray(W2.T / W1_SCALE).astype(np.float32)
    # zero-padded per-tile head blocks: block t covers psum rows 8t..8t+8
    cwh = np.zeros((MLP, 2 * P32 * NT), dtype=np.float32)
    for t in range(NT):
        cwh[:, P32 * t + NH * t:P32 * t + NH * (t + 1)] = Wq.T
        cwh[:, P32 * NT + P32 * t + NH * t:
             P32 * NT + P32 * t + NH * (t + 1)] = Wv.T
    cwh = cwh.astype(ml_dtypes.bfloat16)
    cb = np.stack([b1 * W1_SCALE, b2], axis=1).astype(np.float32)
    pos = np.arange(S, dtype=np.float32)
    maskadd = np.where(mask == 0, np.float32(-1e9), np.float32(0.0))  # (B,S)

    in_maps = []
    for c in range(NCORES):
        sl = slice(c * S_SHARD, (c + 1) * S_SHARD)
        # x-slab layout [p, t*kch + kc, n]: value = x[t, qt*128+n ... ] with
        # contraction row kc*128+p, token index (within shard) split later
        # into quarters by the head matmuls; layer 1 consumes it flat.
        xs = x[:, sl, :].astype(ml_dtypes.float8_e4m3)      # (B, 512, H)
        xt = np.ascontiguousarray(
            xs.reshape(NT, 512, kch, 128).transpose(3, 0, 2, 1))
        # (128, NT, kch, 512) -> [128, NT*kch, 512]
        xt = xt.reshape(128, NT * kch, 512)
        # stats lane p = 32*qt + 8*t + h covers tokens qt*128.. of batch t
        ca2 = np.empty((128, 128 + 1), dtype=np.float32)
        # lane (qt, t, h), col n -> pos qt*128+n, batch t
        posq = pos[sl].reshape(QT, 128)                      # (QT, 128)
        madd = maskadd[:, sl].reshape(NT, QT, 128)           # (T, QT, 128)
        lane_add = (pos_w.astype(np.float32)[None, None, :, None] *
                    posq[None, :, None, :] +
                    madd[:, :, None, :])                     # (T, QT, NH, 128)
        ca2[:, 0:128] = lane_add.reshape(128, 128)
        ca2[:, 128] = np.tile(bv.astype(np.float32), QT * NT)
        in_maps.append({"xt": xt, "w1s": w1s, "cwr": cwr, "cwh": cwh,
                        "cb": cb, "ca2": ca2})
    return in_maps


def merge_stats(stats_all, bias):
    """stats_all: (NCORES, 128, 3), lane 32*qt+8*t+h with [-m, Z, W]
    -> (B, 1) output."""
    st = np.asarray(stats_all, dtype=np.float64).reshape(NCORES, NT, QT, NH, 3)
    st = st.transpose(0, 2, 1, 3, 4).reshape(NCORES * QT, NT, NH, 3)
    m = -st[..., 0]          # (C*QT, B, NH)
    Z = st[..., 1]
    W = st[..., 2]
    M = m.max(axis=0)        # (B, NH)
    alpha = np.exp(m - M[None])
    Zg = (alpha * Z).sum(axis=0)
    Wg = (alpha * W).sum(axis=0)
    out = (Wg / Zg).sum(axis=1)          # (B,)
    return (out[:, None] + np.float64(bias.reshape(1)[0])).astype(np.float32)


def kernel(x, mask, W1, b1, W2, b2, Wq, Wv, bv, pos_w, bias, _trace=False):
    from concourse.bass_utils import run_bass_kernel_spmd

    x = np.asarray(x, dtype=np.float32)
    in_maps = make_core_inputs(x, np.asarray(mask), *(np.asarray(a) for a in
                               (W1, b1, W2, b2, Wq, Wv, bv, pos_w, bias)))
    nc = get_nc()
    res = run_bass_kernel_spmd(nc, in_maps, core_ids=list(range(NCORES)),
                               trace=_trace)
    stats_all = np.stack([r["stats"] for r in res.results])  # (C, 128, 3)
    out = merge_stats(stats_all, np.asarray(bias))
    if _trace:
        kernel.last_result = res
    return out


# revision 36
# speedup vs baseline: 1.0426x; 1.0426x over previous
"""Trainium2 Bass kernel for nn_AttentionProbe_80891414053184.

Math (reference):
    y  = relu(x @ W1.T + b1)            # (B,S,H) -> (B,S,128)
    y2 = relu(y @ W2.T + b2)            # (B,S,128)
    l  = y2 @ Wq.T + pos*pos_w  (+mask) # (B,S,8) logits
    p  = softmax(l, axis=S)
    v  = y2 @ Wv.T + bv
    out[b] = sum_{s,h} p*v + bias       # (B,1)

Strategy: sequence-parallel over 8 cores (512 positions x 4 batches = 2048
tokens per core).  Each core streams its x-shard in fp8-e4m3 (W1 pre-scaled
by 64 into e4m3's normal range, the 1/64 folded into W2; measured end-to-end
error vs the fp32 reference ~5e-3), runs layer 1 as DoubleRow matmuls, the
MLP tail per 512-token tile, and emits per-(batch, head, seq-quarter)
partial softmax stats (-max, Z=sum exp, W=sum exp*v) on all 128 partitions.
The host merges the 8x16 partial stats with the standard online-softmax
combine and produces the (4,1) output.

Layout choices that set the critical path:
  * x is host-pre-swizzled to [128, B*KCH, 512] so every DMA is a
    per-partition-contiguous slab AND the stream is tile-major: tile t's
    k-accumulation completes 1/4 of the way into the remaining stream, so
    its relu/W2/head-projection tail overlaps tile t+1's DMA.  Only tile
    3's tail is exposed at the end.
  * Head projections are 32-wide column-tiled matmuls (N=128) that land
    q|v as a (128, 256) psum: lane p = 32*quarter + 8*tile + head.  The
    whole softmax-stats stage then runs on 128 partitions x 128 columns in
    three fused DVE/ACT ops instead of six 512-wide 32-lane ops.
  * Tail operands (y2, Wq/Wv blocks) are bf16: N=128 matmuls run at
    1 cyc/row in bf16 but 4 cyc/row in f32r.
"""

import numpy as np

# Problem dims (hardcoded per harness contract).
B, S, H = 4, 4096, 4096
MLP, NH = 128, 8
NCORES = 8
S_SHARD = S // NCORES        # 512 seq positions per core
TOK = B * S_SHARD            # 2048 tokens per core
NT = TOK // 512              # 4 token tiles of 512 (= one batch each)
KCH = H // 128               # 32 contraction chunks
GRP = 8                      # k-chunks per x DMA slab (0.5 MB fp8)
NG = KCH // GRP              # 4 slabs per tile
QT = 4                       # seq quarters per tile (512 = 4 x 128)
P32 = 32                     # lanes per psum column group (= NT * NH)

W1_SCALE = 64.0              # 2**6: lifts W1 ~N(0, 1/64^2) into e4m3 range

_cache = {}


def _build_nc(h):
    import concourse.mybir as mybir
    import concourse.tile as tile
    from concourse import bacc
    from concourse.tile import add_dep_helper

    f32 = mybir.dt.float32
    f32r = mybir.dt.float32r
    bf16 = mybir.dt.bfloat16
    fp8 = mybir.dt.float8e4
    kch = h // 128

    # Bacc (not bare Bass): its finalize() runs move_matmul_waits_to_ldweights
    # and generate_event_semaphores, which split multi-sem waits to satisfy
    # TRN2's one-wait-per-instruction encoding limit.
    nc = bacc.Bacc()
    xt_d = nc.dram_tensor("xt", [128, NT * kch, 512], fp8,
                          kind="ExternalInput")
    w1_d = nc.dram_tensor("w1s", [128, kch, MLP], fp8, kind="ExternalInput")
    # cwr: [w2t] f32r for the 512-wide layer-2 matmul; cwh: per-tile
    # zero-padded 32-wide head blocks [wq32 x4 | wv32 x4] bf16 for the
    # 128-wide column-tiled head matmuls; cb: [b1 | b2] f32 biases.
    cwr_d = nc.dram_tensor("cwr", [MLP, MLP], f32r, kind="ExternalInput")
    cwh_d = nc.dram_tensor("cwh", [MLP, 2 * P32 * NT], bf16,
                           kind="ExternalInput")
    cb_d = nc.dram_tensor("cb", [MLP, 2], f32, kind="ExternalInput")
    # ca2: [l-add (pos*pos_w + mask) 128 | bv 1] per stats lane.
    ca2_d = nc.dram_tensor("ca2", [128, 128 + 1], f32, kind="ExternalInput")
    st_d = nc.dram_tensor("stats", [128, 3], f32, kind="ExternalOutput")

    AF = mybir.ActivationFunctionType
    AX = mybir.AxisListType
    OP = mybir.AluOpType
    PM = mybir.MatmulPerfMode.DoubleRow

    with tile.TileContext(nc) as tc:
        with (
            tc.tile_pool(name="const", bufs=1) as const,
            tc.tile_pool(name="xp", bufs=1) as xp,
            tc.tile_pool(name="yp", bufs=2) as yp,
            tc.tile_pool(name="y2p", bufs=2) as y2p,
            tc.tile_pool(name="smallp", bufs=1) as smallp,
            tc.tile_pool(name="statsp", bufs=1) as statsp,
            tc.tile_pool(name="ps_y", bufs=4, space="PSUM") as ps_y,
            tc.tile_pool(name="ps_y2", bufs=1, space="PSUM") as ps_y2,
            tc.tile_pool(name="ps_qv", bufs=1, space="PSUM") as ps_qv,
        ):
            # First stream slab on the HWDGE (sync) ring so it starts during
            # the other engines' preamble, ahead of the SWDGE path.
            x_sb = {}
            def x_dma(t, g, eng):
                sl = xp.tile([128, GRP, 512], fp8, tag=f"x{t}_{g}",
                             name=f"x{t}_{g}")
                eng.dma_start(out=sl[:],
                              in_=xt_d[:, t * kch + g * GRP:
                                       t * kch + (g + 1) * GRP, :])
                x_sb[(t, g)] = sl

            w1_sb = const.tile([128, kch, MLP], fp8)
            h2 = kch // 2
            nc.sync.dma_start(out=w1_sb[:, 0:h2, :], in_=w1_d[:, 0:h2, :])
            x_dma(0, 0, nc.gpsimd)
            x_dma(0, 1, nc.gpsimd)
            nc.sync.dma_start(out=w1_sb[:, h2:kch, :], in_=w1_d[:, h2:kch, :])
            ca2_sb = const.tile([128, 128 + 1], f32)
            nc.scalar.dma_start(out=ca2_sb[:], in_=ca2_d[:])
            cwr_sb = const.tile([MLP, MLP], f32r)
            nc.scalar.dma_start(out=cwr_sb[:], in_=cwr_d[:])
            cwh_sb = const.tile([MLP, 2 * P32 * NT], bf16)
            nc.scalar.dma_start(out=cwh_sb[:], in_=cwh_d[:])
            cb_sb = const.tile([MLP, 2], f32)
            nc.scalar.dma_start(out=cb_sb[:], in_=cb_d[:])

            stats_sb = statsp.tile([128, 3], f32)

            # --- Warmup / staging: each engine observes every const-DMA lane
            # once, so steady-state instructions carry at most one new wait
            # (fewer split-events from Bacc's generate_event_semaphores).
            # Only the w1 warmup gates the k-loop; the rest run after it so
            # the first real matmul waits on nothing but w1-half0 + x-slab0.
            warm_ps = ps_y2.tile([128, 512], f32, tag="y2", name="warm_ps")
            warm_pe_last = nc.tensor.matmul(warm_ps[:, 0:NH], w1_sb[:, 0, :],
                                            w1_sb[:, 0, 0:NH],
                                            start=True, stop=True)

            psum_y = []
            for t in range(NT):
                y_ps = ps_y.tile([128, 512], f32, tag="y", name=f"y_ps{t}")
                psum_y.append(y_ps)

            qv_ps = ps_qv.tile([128, 512], f32, tag="qv", name="qv_ps")

            # cwr/cwh/ca2/cb lane warmups (issued early; they only gate the
            # first tail consumers, not the k-loop)
            warm_ps2 = ps_y2.tile([128, 512], f32, tag="y2", name="warm_ps2")
            nc.tensor.matmul(warm_ps2[0:NH, 0:NH], cwr_sb[:, 0:NH],
                             cwr_sb[:, 0:NH], start=True, stop=True)
            warm_ps3 = ps_y2.tile([128, 512], f32, tag="y2", name="warm_ps3")
            nc.tensor.matmul(warm_ps3[0:NH, 0:NH], cwh_sb[:, 0:NH],
                             cwh_sb[:, 0:NH], start=True, stop=True)
            warm_act = const.tile([MLP, 1], f32)
            nc.scalar.copy(out=warm_act[:], in_=cb_sb[:, 0:1])
            warm_act2 = const.tile([128, 1], f32)
            nc.scalar.copy(out=warm_act2[:], in_=ca2_sb[:, 128:129])
            warm_dve = const.tile([128, 1], f32)
            nc.vector.tensor_copy(out=warm_dve[:], in_=ca2_sb[:, 0:1])
            warm_dve2 = const.tile([MLP, 1], f32)
            nc.vector.tensor_copy(out=warm_dve2[:], in_=cb_sb[:, 1:2])

            # PSUM has_written clears are bank-granular, so the 8 logical
            # head-projection groups (4 column strips x q|v) must share ONE
            # accumulation group: a zero-weight matmul opens it (start=True
            # sets has_written over the full region), every head matmul then
            # accumulates with start=False, and the last carries stop=True.
            zw_sb = const.tile([128, 512], bf16)
            nc.vector.memset(zw_sb[:], 0.0)
            # skip_group_check: the sim's zero-region group checker cannot
            # express column-strip accumulation within one bank; the actual
            # per-element has_written semantics (opener sets all bits, strips
            # accumulate) are still simulated and are what hardware does.
            zero_mm = nc.tensor.matmul(qv_ps[:, 0:512], zw_sb[:, 0:128],
                                       zw_sb[:, 0:512], start=True, stop=False,
                                       skip_group_check=True)

            # Tile-major stream: tile t's layer-1 psum completes after its 4
            # slabs, so its MLP tail overlaps tile t+1's stream.  Layer 1 is
            # DoubleRow fp8: one matmul consumes 2 k-chunks.
            def l1_mms(t, g, last_pair=None):
                sl = x_sb[(t, g)]
                pairs = range(0, GRP, 2) if last_pair is None else [last_pair]
                for kk in pairs:
                    k = g * GRP + kk
                    mm = nc.tensor.matmul(
                        psum_y[t][:],
                        w1_sb[:, k:k + 2, :],
                        sl[:, kk:kk + 2, :],
                        start=(k == 0), stop=(k + 2 == kch),
                        perf_mode=PM,
                    )
                    if k == 0 and t == 0:
                        add_dep_helper(mm.ins, warm_pe_last.ins, sync=False,
                                       reason="warmups before first mm")

            def tail(t):
                # two 256-column halves pipeline relu1 (DVE) -> W2 matmul
                # (PE) -> relu2 (ACT), halving the exposed last-tile latency.
                # Each half's W2 psum is its own bank (psum slots are
                # bank-granular) so the two start/stop groups don't collide.
                y_sb = yp.tile([128, 512], f32r, tag="ysb", name=f"y_sb{t}")
                y2_sb = y2p.tile([128, 512], bf16, tag="y2sb",
                                 name=f"y2_sb{t}")
                for hh in range(2):
                    cs = slice(256 * hh, 256 * (hh + 1))
                    nc.vector.tensor_scalar(out=y_sb[:, cs],
                                            in0=psum_y[t][:, cs],
                                            scalar1=cb_sb[:, 0:1],
                                            scalar2=0.0, op0=OP.add,
                                            op1=OP.max)
                    y2_ps = ps_y2.tile([128, 256], f32, tag=f"y2h{hh}",
                                       name=f"y2_ps{t}_{hh}", bufs=1)
                    nc.tensor.matmul(y2_ps[:], cwr_sb[:], y_sb[:, cs],
                                     start=True, stop=True)
                    nc.scalar.activation(out=y2_sb[:, cs], in_=y2_ps[:],
                                         func=AF.Relu,
                                         bias=cb_sb[:, 1:2], scale=1.0)
                # Head projections: block t is zero outside rows 8t..8t+8, so
                # tiles t=0..3 accumulate into lane group 8t+h of each
                # 32-partition column strip qt; lane p = 32*qt + 8*t + h.
                for qt in range(QT):
                    mm = nc.tensor.matmul(
                        qv_ps[32 * qt:32 * (qt + 1), 0:128],
                        cwh_sb[:, P32 * t:P32 * (t + 1)],
                        y2_sb[:, 128 * qt:128 * (qt + 1)],
                        start=False, stop=False,
                        tile_position=(0, 32 * qt),
                        skip_group_check=True)
                    if t == 0:
                        add_dep_helper(mm.ins, zero_mm.ins, sync=False,
                                       reason="group opener before accum")
                for qt in range(QT):
                    mm = nc.tensor.matmul(
                        qv_ps[32 * qt:32 * (qt + 1), 128:256],
                        cwh_sb[:, P32 * NT + P32 * t:P32 * NT + P32 * (t + 1)],
                        y2_sb[:, 128 * qt:128 * (qt + 1)],
                        start=False, stop=(t == NT - 1),
                        tile_position=(0, 32 * qt),
                        skip_group_check=True)
                    if t == 0:
                        add_dep_helper(mm.ins, zero_mm.ins, sync=False,
                                       reason="group opener before accum")

            for t in range(NT):
                for g in range(NG):
                    if t == NT - 1 and g == NG - 1:
                        # final slab arrives as pair-sized DMAs so the last
                        # tile's matmuls and tail start mid-slab
                        for kk in range(0, GRP, 2):
                            sl = xp.tile([128, 2, 512], fp8,
                                         tag=f"x{t}_{g}_{kk}",
                                         name=f"x{t}_{g}_{kk}")
                            c0 = t * kch + g * GRP + kk
                            nc.gpsimd.dma_start(out=sl[:],
                                                in_=xt_d[:, c0:c0 + 2, :])
                            k = g * GRP + kk
                            nc.tensor.matmul(
                                psum_y[t][:], w1_sb[:, k:k + 2, :], sl[:],
                                start=False, stop=(k + 2 == kch),
                                perf_mode=PM)
                    else:
                        if (t, g) not in x_sb:
                            x_dma(t, g, nc.gpsimd)
                        l1_mms(t, g)
                tail(t)

            # --- softmax stats on (128, 128).
            # (tensor_tensor_reduce would fuse A into one op, but that opcode
            # hard-faults this runtime's DVE -- measured, not theoretical.)
            # A: l = q + add;  stats0 = -max(l)
            l_sb = smallp.tile([128, 128], f32, tag="l", name="l_sb")
            nc.vector.tensor_add(out=l_sb[:], in0=qv_ps[:, 0:128],
                                 in1=ca2_sb[:, 0:128])
            nc.vector.tensor_reduce(out=stats_sb[:, 0:1], in_=l_sb[:],
                                    axis=AX.X, op=OP.max, negate=True)
            # C: e = exp(l - max);  stats1 = Z = sum e
            e_sb = smallp.tile([128, 128], f32, tag="e", name="e_sb")
            nc.scalar.activation(out=e_sb[:], in_=l_sb[:], func=AF.Exp,
                                 bias=stats_sb[:, 0:1], scale=1.0,
                                 accum_out=stats_sb[:, 1:2])
            # D: ev = (v + bv) * e;  stats2 = W = sum ev
            ev_sb = smallp.tile([128, 128], f32, tag="ev", name="ev_sb")
            nc.vector.scalar_tensor_tensor(
                out=ev_sb[:], in0=qv_ps[:, 128:256], scalar=ca2_sb[:, 128:129],
                in1=e_sb[:], op0=OP.add, op1=OP.mult,
                accum_out=stats_sb[:, 2:3])

            nc.sync.dma_start(out=st_d[:], in_=stats_sb[:])

    nc.finalize()
    return nc


def get_nc(h=H):
    if h not in _cache:
        _cache[h] = _build_nc(h)
    return _cache[h]


def make_core_inputs(x, mask, W1, b1, W2, b2, Wq, Wv, bv, pos_w, bias):
    """Host-side shard + transpose. Returns list of 8 in_maps."""
    import ml_dtypes
    h = x.shape[2]
    kch = h // 128
    # W1 scaled up by 64 into e4m3's normal range; layer-1 output then
    # carries a 64x factor, removed by folding 1/64 into W2 (and 64 into b1,
    # since relu(64a) = 64 relu(a) commutes with the positive scale).
    w1s = np.ascontiguousarray(
        (W1 * W1_SCALE).reshape(MLP, kch, 128).transpose(2, 1, 0)).astype(
            ml_dtypes.float8_e4m3)
    cwr = np.ascontiguousarray(W2.T / W1_SCALE).astype(np.float32)
    # zero-padded per-tile head blocks: block t covers psum rows 8t..8t+8
    cwh = np.zeros((MLP, 2 * P32 * NT), dtype=np.float32)
    for t in range(NT):
        cwh[:, P32 * t + NH * t:P32 * t + NH * (t + 1)] = Wq.T
        cwh[:, P32 * NT + P32 * t + NH * t:
             P32 * NT + P32 * t + NH * (t + 1)] = Wv.T
    cwh = cwh.astype(ml_dtypes.bfloat16)
    cb = np.stack([b1 * W1_SCALE, b2], axis=1).astype(np.float32)
    pos = np.arange(S, dtype=np.float32)
    maskadd = np.where(mask == 0, np.float32(-1e9), np.float32(0.0))  # (B,S)

    in_maps = []
    for c in range(NCORES):
        sl = slice(c * S_SHARD, (c + 1) * S_SHARD)
        # x-slab layout [p, t*kch + kc, n]: value = x[t, qt*128+n ... ] with
        # contraction row kc*128+p, token index (within shard) split later
        # into quarters by the head matmuls; layer 1 consumes it flat.
        xs = x[:, sl, :].astype(ml_dtypes.float8_e4m3)      # (B, 512, H)
        xt = np.ascontiguousarray(
            xs.reshape(NT, 512, kch, 128).transpose(3, 0, 2, 1))
        # (128, NT, kch, 512) -> [128, NT*kch, 512]
        xt = xt.reshape(128, NT * kch, 512)
        # stats lane p = 32*qt + 8*t + h covers tokens qt*128.. of batch t
        ca2 = np.empty((128, 128 + 1), dtype=np.float32)
        # lane (qt, t, h), col n -> pos qt*128+n, batch t
        posq = pos[sl].reshape(QT, 128)                      # (QT, 128)
        madd = maskadd[:, sl].reshape(NT, QT, 128)           # (T, QT, 128)
        lane_add = (pos_w.astype(np.float32)[None, None, :, None] *
                    posq[:, None, None, :] +
                    madd.transpose(1, 0, 2)[:, :, None, :])  # (QT, T, NH, 128)
        ca2[:, 0:128] = lane_add.reshape(128, 128)
        ca2[:, 128] = np.tile(bv.astype(np.float32), QT * NT)
        in_maps.append({"xt": xt, "w1s": w1s, "cwr": cwr, "cwh": cwh,
                        "cb": cb, "ca2": ca2})
    return in_maps


def merge_stats(stats_all, bias):
    """stats_all: (NCORES, 128, 3), lane 32*qt+8*t+h with [-m, Z, W]
    -> (B, 1) output."""
    st = np.asarray(stats_all, dtype=np.float64).reshape(NCORES * QT, NT, NH, 3)
    m = -st[..., 0]          # (C*QT, B, NH)
    Z = st[..., 1]
    W = st[..., 2]
    M = m.max(axis=0)        # (B, NH)
    alpha = np.exp(m - M[None])
    Zg = (alpha * Z).sum(axis=0)
    Wg = (alpha * W).sum(axis=0)
    out = (Wg / Zg).sum(axis=1)          # (B,)
    return (out[:, None] + np.float64(bias.reshape(1)[0])).astype(np.float32)


def kernel(x, mask, W1, b1, W2, b2, Wq, Wv, bv, pos_w, bias, _trace=False):
    from concourse.bass_utils import run_bass_kernel_spmd

    x = np.asarray(x, dtype=np.float32)
    in_maps = make_core_inputs(x, np.asarray(mask), *(np.asarray(a) for a in
                               (W1, b1, W2, b2, Wq, Wv, bv, pos_w, bias)))
    nc = get_nc()
    res = run_bass_kernel_spmd(nc, in_maps, core_ids=list(range(NCORES)),
                               trace=_trace)
    stats_all = np.stack([r["stats"] for r in res.results])  # (C, 128, 3)
    out = merge_stats(stats_all, np.asarray(bias))
    if _trace:
        kernel.last_result = res
    return out
